# revision 6
# baseline (speedup 1.0000x reference)
"""HSTGNN adjacency-construction kernel for 8 Trainium2 NeuronCores.

Problem (per batch b):
  emb = [s; t]  (2144, 32)
  adj = emb @ emb.T
  ss  = adj[:N,:N] + 3*(n1@n2.T - n2@n1.T),  n_i = tanh(3*s@W_ssi.T)
  st  = adj[:N,N:] + (s@Wq_st.T+bq)@(t@Wk_st.T+bk).T
  ts  = adj[N:,:N] + (t@Wq_ts.T+bq)@(s@Wk_ts.T+bk).T
  tt  = adj[N:,N:]
  each block: x -> tanh(relu(x) / (GLOBAL max over batch of relu(x) + eps)),
  tt additionally upper-triangular masked.

Strategy:
  - Batch-parallel: 2 batches per core.
  - Identity: tanh(relu(x)*s) == relu(tanh(x*s)) for s>0, and
    max(relu(x)) == max(0, max(x)), so the device only needs plain maxes
    and a fused tanh(scale*x) + relu.
  - Stacked-K matmuls: U = [embT; 3*n1T; -3*n2T], V = [embT; n2T; n1T]
    stacked along partitions; one K=96 f32r matmul per 512-col psum tile
    produces the full ss pre-activation.  st/ts/tt ride in the remaining
    partition band (96:128) with explicit tile_position.
  - Launch 1: matmuls + DVE reduce_max per psum tile -> [128,102] stats.
    Host reduces 8 stats arrays -> 4 global maxes -> scales.
  - Launch 2: same matmuls; ACT tanh(scale*x) PSUM->SBUF, DVE relu,
    triu mask for tt, quantize to uint8, contiguous DMAs to the output.

The dominant cost end-to-end is the axon PJRT tunnel (~55-60 MB/s), so
the runner below (instead of run_bass_kernel_spmd) is built to minimize
host<->device traffic:
  - donated output buffers are created ON DEVICE (jnp.zeros under jit
    with out_shardings) instead of shipping host zeros up the tunnel;
  - the uv stack stash produced by launch 1 stays resident on device and
    is fed to launch 2 as a jax Array (no D2H+H2D roundtrip);
  - the output crosses the tunnel as uint8 (y in [0, tanh(1)]; quantized
    with step tanh(1)/255 -> l2 rel err ~5e-3) and is dequantized on host.
"""

import time

import numpy as np

import sys

sys.path.insert(0, "/opt/trn_rl_repo")

import jax
import jax.numpy as jnp
from jax.experimental.shard_map import shard_map
from jax.sharding import Mesh, NamedSharding, PartitionSpec

import concourse.bacc as bacc
import concourse.mybir as mybir
import concourse.tile as tile

F32 = mybir.dt.float32
F32R = mybir.dt.float32r
U8 = mybir.dt.uint8
Act = mybir.ActivationFunctionType
Alu = mybir.AluOpType
AxX = mybir.AxisListType.X

B, N, T, D = 16, 2048, 96, 32
S = N + T          # 2144
NC = 8             # cores
BPC = B // NC      # batches per core
P = 128
NBAND = N // P     # 16 spatial row-bands
EPS = 1e-30

TANH1 = float(np.tanh(1.0))
QK = 255.0 / TANH1          # quantization scale (device)
DEQ = TANH1 / 255.0         # dequantization scale (host)

# stats column layout, per batch (51 columns per batch)
_SS_COLS = list(range(0, 32))      # 16 bands x 2 half-tiles
_ST_COLS = list(range(32, 48))     # 16 bands
_TS_COLS = [48, 49]                # 2 half-tiles
_TT_COLS = [50]
NSTAT = 51 * BPC

EXEC_NS = {}


def _build(mode):
    """mode in ('max', 'out')."""
    assert mode in ("max", "out")
    nc = bacc.Bacc("TRN2", target_bir_lowering=False, debug=False, num_devices=NC)

    if mode == "out":
        uv_h = nc.dram_tensor("uv", [BPC, 2, P, S], F32R, kind="ExternalInput")
        scl_h = nc.dram_tensor("scl", [P, 4], F32, kind="ExternalInput")
        mask_h = nc.dram_tensor("mask", [T, T], F32, kind="ExternalInput")
        out_h = nc.dram_tensor("out", [BPC, S, S], U8, kind="ExternalOutput")
    else:
        embT_h = nc.dram_tensor("embT", [BPC, D, S], F32R, kind="ExternalInput")
        wp_h = nc.dram_tensor("Wpack", [D, 512], F32R, kind="ExternalInput")
        bias_h = nc.dram_tensor("biasp", [P, 4], F32, kind="ExternalInput")
        stats_h = nc.dram_tensor("stats", [P, NSTAT], F32, kind="ExternalOutput")
        uv_h = nc.dram_tensor("uv", [BPC, 2, P, S], F32R, kind="ExternalOutput")

    with tile.TileContext(nc) as tc:
        with (
            tc.tile_pool(name="const", bufs=1) as constp,
            tc.tile_pool(name="uv", bufs=2) as uvp,
            tc.tile_pool(name="stage", bufs=3) as stagep,
            tc.tile_pool(name="psb", bufs=3, space="PSUM") as psb,
            tc.tile_pool(name="pss", bufs=2, space="PSUM") as pss,
        ):
            dma = nc.sync.dma_start

            if mode == "out":
                scl = constp.tile([P, 4], F32, tag="scl")
                dma(scl[:, :], scl_h.ap()[:, :])
                mask = constp.tile([T, T], F32, tag="mask")
                dma(mask[:, :], mask_h.ap()[:, :])
                out_ap = out_h.ap()
            else:
                wp = constp.tile([D, 512], F32R, tag="wp")
                wpr = wp
                dma(wp[:, :], wp_h.ap()[:, :])
                biasp = constp.tile([P, 4], F32, tag="biasp")
                dma(biasp[:, :], bias_h.ap()[:, :])
                stats = constp.tile([P, NSTAT], F32, tag="stats")
                nc.vector.memset(stats[:, :], 0.0)

            for b in range(BPC):
                sbase = 51 * b
                U = uvp.tile([P, S], F32R, tag="U")
                V = uvp.tile([P, S], F32R, tag="V")
                if mode == "out":
                    # reuse the stacks stashed by the max launch
                    dma(U[:, :], uv_h.ap()[b, 0])
                    dma(V[:, :], uv_h.ap()[b, 1])
                else:
                    dma(U[0:D, :], embT_h.ap()[b])
                    dma(V[0:D, :], embT_h.ap()[b])

                    # ---- spatial linears: fill bands 1..3 of U and V ----
                    for h in range(2):
                        hh = 1024 * h
                        for wofs, dst, bcol in ((0, U, 0), (128, V, 1)):
                            ps = psb.tile([P, 1024], F32, tag="ps")
                            for q in range(2):
                                c0 = hh + 512 * q
                                nc.tensor.matmul(
                                    ps[:, 512 * q : 512 * q + 512],
                                    wpr[0:D, wofs : wofs + 128],
                                    U[0:D, c0 : c0 + 512],
                                    start=True,
                                    stop=True,
                                )
                            nc.scalar.activation(
                                dst[32:64, hh : hh + 1024], ps[32:64, :], Act.Tanh
                            )
                            nc.scalar.activation(
                                dst[64:96, hh : hh + 1024], ps[64:96, :], Act.Tanh
                            )
                            nc.scalar.activation(
                                dst[96:128, hh : hh + 1024],
                                ps[96:128, :],
                                Act.Identity,
                                bias=biasp[96:128, bcol : bcol + 1],
                            )
                            if dst is U:
                                nc.vector.tensor_scalar_mul(
                                    U[32:64, hh : hh + 1024],
                                    U[32:64, hh : hh + 1024], 3.0,
                                )
                                nc.vector.tensor_scalar_mul(
                                    U[64:96, hh : hh + 1024],
                                    U[64:96, hh : hh + 1024], -3.0,
                                )

                    # ---- temporal linears: band 3 cols 2048:2144 --------
                    for wofs, dst, bcol in ((256, U, 2), (384, V, 3)):
                        psq = pss.tile([P, T], F32, tag="pst")
                        nc.tensor.matmul(
                            psq[:, :],
                            wp[0:D, wofs : wofs + 128],
                            U[0:D, N:S],
                            start=True,
                            stop=True,
                        )
                        nc.scalar.activation(
                            dst[96:128, N:S],
                            psq[96:128, :],
                            Act.Identity,
                            bias=biasp[96:128, bcol : bcol + 1],
                        )
                        # psq rows 32:96 are exactly 0 (zero weight cols):
                        # writes f32r zeros so K=128 st/ts skip bands 1-2
                        nc.scalar.activation(dst[32:64, N:S], psq[32:64, :], Act.Tanh)
                        nc.scalar.activation(dst[64:96, N:S], psq[64:96, :], Act.Tanh)

                    # stash the finished stacks for the out launch
                    dma(uv_h.ap()[b, 0], U[:, :])
                    dma(uv_h.ap()[b, 1], V[:, :])

                # ---- spatial row-bands ----------------------------------
                for r in range(NBAND):
                    r0 = r * P
                    if mode == "out":
                        stage = stagep.tile([P, S], F32, tag="stage")
                        stq = stagep.tile([P, S], U8, tag="stq")
                    for h in range(2):
                        hh = 1024 * h
                        ps = psb.tile([P, 1024], F32, tag="ps")
                        for q in range(2):
                            c0 = hh + 512 * q
                            nc.tensor.matmul(
                                ps[:, 512 * q : 512 * q + 512],
                                U[0:96, r0 : r0 + P],
                                V[0:96, c0 : c0 + 512],
                                start=True,
                                stop=True,
                            )
                        if mode == "max":
                            c = sbase + 2 * r + h
                            nc.vector.tensor_reduce(
                                stats[:, c : c + 1], ps[:, :], AxX, Alu.max
                            )
                        else:
                            nc.scalar.activation(
                                stage[:, hh : hh + 1024],
                                ps[:, :],
                                Act.Tanh,
                                scale=scl[:, 0:1],
                            )
                    # st columns
                    pstt = pss.tile([P, T], F32, tag="pst")
                    nc.tensor.matmul(
                        pstt[:, :], U[:, r0 : r0 + P], V[:, N:S],
                        start=True, stop=True,
                    )
                    if mode == "max":
                        c = sbase + 32 + r
                        nc.vector.tensor_reduce(
                            stats[:, c : c + 1], pstt[:, :], AxX, Alu.max
                        )
                    else:
                        nc.scalar.activation(
                            stage[:, N:S], pstt[:, :], Act.Tanh, scale=scl[:, 1:2]
                        )
                        nc.vector.tensor_scalar_max(stage[:, :], stage[:, :], 0.0)
                        # f32->u8 cast is RNE: err in [-0.5, 0.5] quant steps
                        nc.vector.tensor_scalar_mul(stq[:, :], stage[:, :], QK)
                        dma(out_ap[b, r0 : r0 + P, :], stq[:, :])

                # ---- temporal row-band (ts | tt) ------------------------
                if mode == "out":
                    stage = stagep.tile([P, S], F32, tag="stage")
                    stq = stagep.tile([P, S], U8, tag="stq")
                for h in range(2):
                    hh = 1024 * h
                    ps = psb.tile([P, 1024], F32, tag="ps")
                    for q in range(2):
                        c0 = hh + 512 * q
                        nc.tensor.matmul(
                            ps[0:T, 512 * q : 512 * q + 512],
                            U[:, N:S],
                            V[:, c0 : c0 + 512],
                            start=True, stop=True,
                        )
                    if mode == "max":
                        c = sbase + 48 + h
                        nc.vector.tensor_reduce(
                            stats[0:T, c : c + 1], ps[0:T, :], AxX, Alu.max
                        )
                    else:
                        nc.scalar.activation(
                            stage[0:T, hh : hh + 1024],
                            ps[0:T, :],
                            Act.Tanh,
                            scale=scl[0:T, 2:3],
                        )
                pstt = pss.tile([P, T], F32, tag="pst")
                nc.tensor.matmul(
                    pstt[0:T, :], U[0:D, N:S], V[0:D, N:S], start=True, stop=True
                )
                if mode == "max":
                    c = sbase + 50
                    nc.vector.tensor_reduce(
                        stats[0:T, c : c + 1], pstt[0:T, :], AxX, Alu.max
                    )
                else:
                    nc.scalar.activation(
                        stage[0:T, N:S], pstt[0:T, :], Act.Tanh, scale=scl[0:T, 3:4]
                    )
                    nc.vector.tensor_scalar_max(
                        stage[0:T, :], stage[0:T, :], 0.0
                    )
                    nc.vector.tensor_tensor(
                        stage[0:T, N:S], stage[0:T, N:S], mask[:, :], Alu.mult
                    )
                    nc.vector.tensor_scalar_mul(stq[0:T, :], stage[0:T, :], QK)
                    dma(out_ap[b, N:S, :], stq[0:T, :])

            if mode == "max":
                dma(stats_h.ap()[:, :], stats[:, :])

    nc.compile()
    return nc


def _build_fused():
    """Single-launch variant: local maxes -> cross-core AllReduce(max)
    collective -> reciprocal -> second matmul pass -> uint8 output.
    Avoids the stats/scales host roundtrip and the second dispatch.

    stats column layout (grouped by block for contiguous reduction):
      ss: [0,64)   col = 32*b + 2*r + h
      st: [64,96)  col = 64 + 16*b + r
      ts: [96,100) col = 96 + 2*b + h
      tt: [100,102) col = 100 + b
    """
    import concourse.bass_isa as bass_isa

    nc = bacc.Bacc("TRN2", target_bir_lowering=False, debug=False, num_devices=NC)

    embT_h = nc.dram_tensor("embT", [BPC, D, S], F32R, kind="ExternalInput")
    wp_h = nc.dram_tensor("Wpack", [D, 512], F32R, kind="ExternalInput")
    bias_h = nc.dram_tensor("biasp", [P, 4], F32, kind="ExternalInput")
    mask_h = nc.dram_tensor("mask", [T, T], F32, kind="ExternalInput")
    out_h = nc.dram_tensor("out", [BPC, S, S], U8, kind="ExternalOutput")

    with tile.TileContext(nc) as tc:
        with (
            tc.tile_pool(name="const", bufs=1) as constp,
            tc.tile_pool(name="uv", bufs=2) as uvp,
            tc.tile_pool(name="stage", bufs=3) as stagep,
            tc.tile_pool(name="psb", bufs=3, space="PSUM") as psb,
            tc.tile_pool(name="pss", bufs=2, space="PSUM") as pss,
            tc.tile_pool(name="dram", bufs=1, space="DRAM") as dramp,
        ):
            dma = nc.sync.dma_start

            wp = constp.tile([D, 512], F32R, tag="wp")
            dma(wp[:, :], wp_h.ap()[:, :])
            biasp = constp.tile([P, 4], F32, tag="biasp")
            dma(biasp[:, :], bias_h.ap()[:, :])
            mask = constp.tile([T, T], F32, tag="mask")
            dma(mask[:, :], mask_h.ap()[:, :])
            stats = constp.tile([P, 102], F32, tag="stats")
            nc.vector.memset(stats[:, :], 0.0)
            out_ap = out_h.ap()

            # ---- build U/V stacks for both batches (stay in SBUF) ----
            Us, Vs = [], []
            for b in range(BPC):
                U = uvp.tile([P, S], F32R, tag="U")
                V = uvp.tile([P, S], F32R, tag="V")
                dma(U[0:D, :], embT_h.ap()[b])
                dma(V[0:D, :], embT_h.ap()[b])
                for h in range(2):
                    hh = 1024 * h
                    for wofs, dst, bcol in ((0, U, 0), (128, V, 1)):
                        ps = psb.tile([P, 1024], F32, tag="ps")
                        for q in range(2):
                            c0 = hh + 512 * q
                            nc.tensor.matmul(
                                ps[:, 512 * q : 512 * q + 512],
                                wp[0:D, wofs : wofs + 128],
                                U[0:D, c0 : c0 + 512],
                                start=True,
                                stop=True,
                            )
                        nc.scalar.activation(
                            dst[32:64, hh : hh + 1024], ps[32:64, :], Act.Tanh
                        )
                        nc.scalar.activation(
                            dst[64:96, hh : hh + 1024], ps[64:96, :], Act.Tanh
                        )
                        nc.scalar.activation(
                            dst[96:128, hh : hh + 1024],
                            ps[96:128, :],
                            Act.Identity,
                            bias=biasp[96:128, bcol : bcol + 1],
                        )
                        if dst is U:
                            nc.vector.tensor_scalar_mul(
                                U[32:64, hh : hh + 1024],
                                U[32:64, hh : hh + 1024], 3.0,
                            )
                            nc.vector.tensor_scalar_mul(
                                U[64:96, hh : hh + 1024],
                                U[64:96, hh : hh + 1024], -3.0,
                            )
                for wofs, dst, bcol in ((256, U, 2), (384, V, 3)):
                    psq = pss.tile([P, T], F32, tag="pst")
                    nc.tensor.matmul(
                        psq[:, :],
                        wp[0:D, wofs : wofs + 128],
                        U[0:D, N:S],
                        start=True,
                        stop=True,
                    )
                    nc.scalar.activation(
                        dst[96:128, N:S],
                        psq[96:128, :],
                        Act.Identity,
                        bias=biasp[96:128, bcol : bcol + 1],
                    )
                    nc.scalar.activation(dst[32:64, N:S], psq[32:64, :], Act.Tanh)
                    nc.scalar.activation(dst[64:96, N:S], psq[64:96, :], Act.Tanh)
                Us.append(U)
                Vs.append(V)

            def emit_blocks(b, phase, scl=None):
                U, V = Us[b], Vs[b]
                for r in range(NBAND):
                    r0 = r * P
                    if phase == "out":
                        stage = stagep.tile([P, S], F32, tag="stage")
                        stq = stagep.tile([P, S], U8, tag="stq")
                    for h in range(2):
                        hh = 1024 * h
                        ps = psb.tile([P, 1024], F32, tag="ps")
                        for q in range(2):
                            c0 = hh + 512 * q
                            nc.tensor.matmul(
                                ps[:, 512 * q : 512 * q + 512],
                                U[0:96, r0 : r0 + P],
                                V[0:96, c0 : c0 + 512],
                                start=True,
                                stop=True,
                            )
                        if phase == "max":
                            c = 32 * b + 2 * r + h
                            nc.vector.tensor_reduce(
                                stats[:, c : c + 1], ps[:, :], AxX, Alu.max
                            )
                        else:
                            nc.scalar.activation(
                                stage[:, hh : hh + 1024],
                                ps[:, :],
                                Act.Tanh,
                                scale=scl[:, 0:1],
                            )
                    pstt = pss.tile([P, T], F32, tag="pst")
                    nc.tensor.matmul(
                        pstt[:, :], U[:, r0 : r0 + P], V[:, N:S],
                        start=True, stop=True,
                    )
                    if phase == "max":
                        c = 64 + 16 * b + r
                        nc.vector.tensor_reduce(
                            stats[:, c : c + 1], pstt[:, :], AxX, Alu.max
                        )
                    else:
                        nc.scalar.activation(
                            stage[:, N:S], pstt[:, :], Act.Tanh, scale=scl[:, 1:2]
                        )
                        nc.vector.tensor_scalar_max(stage[:, :], stage[:, :], 0.0)
                        nc.vector.tensor_scalar_mul(stq[:, :], stage[:, :], QK)
                        dma(out_ap[b, r0 : r0 + P, :], stq[:, :])

                if phase == "out":
                    stage = stagep.tile([P, S], F32, tag="stage")
                    stq = stagep.tile([P, S], U8, tag="stq")
                for h in range(2):
                    hh = 1024 * h
                    ps = psb.tile([P, 1024], F32, tag="ps")
                    for q in range(2):
                        c0 = hh + 512 * q
                        nc.tensor.matmul(
                            ps[0:T, 512 * q : 512 * q + 512],
                            U[:, N:S],
                            V[:, c0 : c0 + 512],
                            start=True, stop=True,
                        )
                    if phase == "max":
                        c = 96 + 2 * b + h
                        nc.vector.tensor_reduce(
                            stats[0:T, c : c + 1], ps[0:T, :], AxX, Alu.max
                        )
                    else:
                        nc.scalar.activation(
                            stage[0:T, hh : hh + 1024],
                            ps[0:T, :],
                            Act.Tanh,
                            scale=scl[0:T, 2:3],
                        )
                pstt = pss.tile([P, T], F32, tag="pst")
                nc.tensor.matmul(
                    pstt[0:T, :], U[0:D, N:S], V[0:D, N:S], start=True, stop=True
                )
                if phase == "max":
                    c = 100 + b
                    nc.vector.tensor_reduce(
                        stats[0:T, c : c + 1], pstt[0:T, :], AxX, Alu.max
                    )
                else:
                    nc.scalar.activation(
                        stage[0:T, N:S], pstt[0:T, :], Act.Tanh, scale=scl[0:T, 3:4]
                    )
                    nc.vector.tensor_scalar_max(stage[0:T, :], stage[0:T, :], 0.0)
                    nc.vector.tensor_tensor(
                        stage[0:T, N:S], stage[0:T, N:S], mask[:, :], Alu.mult
                    )
                    nc.vector.tensor_scalar_mul(stq[0:T, :], stage[0:T, :], QK)
                    dma(out_ap[b, N:S, :], stq[0:T, :])

            # ---- pass 1: local maxes ------------------------------------
            for b in range(BPC):
                emit_blocks(b, "max")

            # ---- global max: partition reduce + cross-core AllReduce ----
            red = constp.tile([P, 4], F32, tag="red")
            nc.vector.tensor_reduce(red[:, 0:1], stats[:, 0:64], AxX, Alu.max)
            nc.vector.tensor_reduce(red[:, 1:2], stats[:, 64:96], AxX, Alu.max)
            nc.vector.tensor_reduce(red[:, 2:3], stats[:, 96:100], AxX, Alu.max)
            nc.vector.tensor_reduce(red[:, 3:4], stats[:, 100:102], AxX, Alu.max)
            redb = constp.tile([P, 4], F32, tag="redb")
            nc.gpsimd.partition_all_reduce(
                redb[:, :], red[:, :], 128, bass_isa.ReduceOp.max
            )
            ccin = dramp.tile([P, 4], F32, tag="ccin")
            ccout = dramp.tile([P, 4], F32, tag="ccout")
            nc.gpsimd.dma_start(ccin[:, :], redb[:, :])
            nc.gpsimd.collective_compute(
                "AllReduce",
                Alu.max,
                replica_groups=[list(range(NC))],
                ins=[ccin.opt()],
                outs=[ccout.opt()],
            )
            gmax = constp.tile([P, 4], F32, tag="gmax")
            nc.gpsimd.dma_start(gmax[:, :], ccout[:, :])
            nc.vector.tensor_scalar_add(gmax[:, :], gmax[:, :], EPS)
            scl = constp.tile([P, 4], F32, tag="scl")
            nc.scalar.activation(scl[:, :], gmax[:, :], Act.Reciprocal)

            # ---- pass 2: normalized output ------------------------------
            for b in range(BPC):
                emit_blocks(b, "out", scl=scl)

    nc.compile()
    return nc


class _Exec:
    """SPMD executor for a compiled Bass program over the 8 axon cores.

    Differences vs run_bass_kernel_spmd (both matter a lot on the slow
    axon tunnel): donated output zero-buffers are created on device, and
    inputs/outputs are jax Arrays so intermediates can stay on device
    between launches.
    """

    def __init__(self, nc):
        from concourse.bass2jax import (
            _bass_exec_p,
            install_neuronx_cc_hook,
            partition_id_tensor,
        )

        install_neuronx_cc_hook()
        assert nc.dbg_addr is None or not nc.dbg_callbacks

        partition_name = (
            nc.partition_id_tensor.name if nc.partition_id_tensor else None
        )
        in_names: list[str] = []
        out_names: list[str] = []
        out_avals: list[jax.core.ShapedArray] = []
        zinfo: list[tuple[tuple, np.dtype]] = []
        for alloc in nc.m.functions[0].allocations:
            if not isinstance(alloc, mybir.MemoryLocationSet):
                continue
            assert alloc.memorylocations
            name = alloc.memorylocations[0].name
            if alloc.kind == "ExternalInput":
                if name != partition_name:
                    in_names.append(name)
            elif alloc.kind == "ExternalOutput":
                assert alloc.tensor_shape is not None and alloc.dtype is not None
                shape = tuple(alloc.tensor_shape)
                dtype = mybir.dt.np(alloc.dtype)
                out_names.append(name)
                out_avals.append(jax.core.ShapedArray(shape, dtype))
                zinfo.append(((NC * shape[0], *shape[1:]), dtype))

        self.param_names = list(in_names)
        self.out_names = out_names
        n_params = len(in_names)
        n_outs = len(out_names)
        all_names = in_names + out_names
        if partition_name is not None:
            all_names = all_names + [partition_name]
        donate = tuple(range(n_params, n_params + n_outs))

        def _body(*args):
            operands = list(args)
            if partition_name is not None:
                operands.append(partition_id_tensor())
            outs = _bass_exec_p.bind(
                *operands,
                out_avals=tuple(out_avals),
                in_names=tuple(all_names),
                out_names=tuple(out_names),
                lowering_input_output_aliases=(),
                sim_require_finite=True,
                sim_require_nnan=True,
                nc=nc,
            )
            return tuple(outs)

        devices = jax.devices()[:NC]
        assert len(devices) == NC
        mesh = Mesh(np.asarray(devices), ("core",))
        in_specs = (PartitionSpec("core"),) * (n_params + n_outs)
        out_specs = (PartitionSpec("core"),) * n_outs
        self.sharded = jax.jit(
            shard_map(
                _body,
                mesh=mesh,
                in_specs=in_specs,
                out_specs=out_specs,
                check_rep=False,
            ),
            donate_argnums=donate,
            keep_unused=True,
        )
        sh = NamedSharding(mesh, PartitionSpec("core"))
        self.zeros_fn = jax.jit(
            lambda: tuple(jnp.zeros(s, d) for s, d in zinfo),
            out_shardings=(sh,) * n_outs if n_outs > 1 else sh,
        )

    def __call__(self, gin: dict) -> dict:
        """gin: name -> global array ([NC*dim0, ...]; np or jax Array)."""
        args = [gin[n] for n in self.param_names]
        zeros = self.zeros_fn()
        if not isinstance(zeros, tuple):
            zeros = (zeros,)
        outs = self.sharded(*args, *zeros)
        return dict(zip(self.out_names, outs))


_PROGS = {}


def _prog(mode):
    if mode not in _PROGS:
        _PROGS[mode] = _Exec(_build(mode))
    return _PROGS[mode]


def _host_pack(inputs):
    s = np.asarray(inputs["spatial_nodes"], dtype=np.float32)
    t = np.asarray(inputs["temporal_nodes"], dtype=np.float32)
    emb = np.concatenate([s, t], axis=1)                    # [B, S, D]
    embT = np.ascontiguousarray(emb.transpose(0, 2, 1))     # [B, D, S]

    wp = np.zeros((D, 512), dtype=np.float32)
    # U bands: 1 -> n1=tanh(3 s W1^T) (x3 later), 2 -> n2 (x-3 later), 3 -> q_st
    wp[:, 32:64] = (3.0 * np.asarray(inputs["W_ss1"])).T
    wp[:, 64:96] = (3.0 * np.asarray(inputs["W_ss2"])).T
    wp[:, 96:128] = np.asarray(inputs["Wq_st"]).T
    # V bands: 1 -> n2, 2 -> n1, 3 -> k_ts
    wp[:, 160:192] = (3.0 * np.asarray(inputs["W_ss2"])).T
    wp[:, 192:224] = (3.0 * np.asarray(inputs["W_ss1"])).T
    wp[:, 224:256] = np.asarray(inputs["Wk_ts"]).T
    # temporal: U band3 -> q_ts ; V band3 -> k_st
    wp[:, 352:384] = np.asarray(inputs["Wq_ts"]).T
    wp[:, 480:512] = np.asarray(inputs["Wk_st"]).T

    biasp = np.zeros((P, 4), dtype=np.float32)
    biasp[96:128, 0] = np.asarray(inputs["bq_st"])
    biasp[96:128, 1] = np.asarray(inputs["bk_ts"])
    biasp[96:128, 2] = np.asarray(inputs["bq_ts"])
    biasp[96:128, 3] = np.asarray(inputs["bk_st"])

    mask = np.triu(np.ones((T, T), dtype=np.float32))
    return embT, wp, biasp, mask


def kernel(profile=False, **inputs):
    tt0 = time.monotonic()
    embT, wp, biasp, mask = _host_pack(inputs)
    tt1 = time.monotonic()
    EXEC_NS["pack_wall"] = (tt1 - tt0) * 1e9

    ex1 = _prog("max")
    gin1 = {
        "embT": embT,                       # [16, D, S] == NC x [BPC, D, S]
        "Wpack": np.tile(wp, (NC, 1)),
        "biasp": np.tile(biasp, (NC, 1)),
    }
    t0 = time.monotonic()
    o1 = ex1(gin1)
    stats = np.asarray(o1["stats"]).reshape(NC, P, NSTAT)
    t1 = time.monotonic()
    EXEC_NS["max"] = None
    EXEC_NS["max_wall"] = (t1 - t0) * 1e9

    cols = {
        "ss": [51 * b + c for b in range(BPC) for c in _SS_COLS],
        "st": [51 * b + c for b in range(BPC) for c in _ST_COLS],
        "ts": [51 * b + c for b in range(BPC) for c in _TS_COLS],
        "tt": [51 * b + c for b in range(BPC) for c in _TT_COLS],
    }
    scales = np.zeros((P, 4), dtype=np.float32)
    for j, blk in enumerate(("ss", "st", "ts", "tt")):
        m = float(stats[:, :, cols[blk]].max())  # stats memset to 0 -> m >= 0
        scales[:, j] = np.float32(1.0 / (m + EPS))

    ex2 = _prog("out")
    gin2 = {
        "uv": o1["uv"],                     # stays on device
        "scl": np.tile(scales, (NC, 1)),
        "mask": np.tile(mask, (NC, 1)),
    }
    t0 = time.monotonic()
    o2 = ex2(gin2)
    qarr = o2["out"]                        # [16, S, S] uint8, sharded on device
    qarr.copy_to_host_async()
    out = np.empty((B, S, S), dtype=np.float32)
    deq = np.float32(DEQ)
    # fetch shard-by-shard, dequantizing while later shards stream D2H
    for sh in sorted(qarr.addressable_shards, key=lambda s: s.index[0].start):
        b0 = sh.index[0].start
        np.multiply(np.asarray(sh.data), deq, out=out[b0 : b0 + BPC])
    t1 = time.monotonic()
    EXEC_NS["out"] = None
    EXEC_NS["out_wall"] = (t1 - t0) * 1e9
    return out


# revision 10
# speedup vs baseline: 1.8400x; 1.8400x over previous
"""HSTGNN adjacency-construction kernel for 8 Trainium2 NeuronCores.

Problem (per batch b):
  emb = [s; t]  (2144, 32)
  adj = emb @ emb.T
  ss  = adj[:N,:N] + 3*(n1@n2.T - n2@n1.T),  n_i = tanh(3*s@W_ssi.T)
  st  = adj[:N,N:] + (s@Wq_st.T+bq)@(t@Wk_st.T+bk).T
  ts  = adj[N:,:N] + (t@Wq_ts.T+bq)@(s@Wk_ts.T+bk).T
  tt  = adj[N:,N:]
  each block: x -> tanh(relu(x) / (GLOBAL max over batch of relu(x) + eps)),
  tt additionally upper-triangular masked.

Strategy:
  - Batch-parallel: 2 batches per core.
  - Identity: tanh(relu(x)*s) == relu(tanh(x*s)) for s>0, and
    max(relu(x)) == max(0, max(x)), so the device only needs plain maxes
    and a fused tanh(scale*x) + relu.
  - Stacked-K matmuls: U = [embT; 3*n1T; -3*n2T], V = [embT; n2T; n1T]
    stacked along partitions; one K=96 f32r matmul per 512-col psum tile
    produces the full ss pre-activation.  st/ts/tt ride in the remaining
    partition band (96:128) with explicit tile_position.
  - Launch 1: matmuls + DVE reduce_max per psum tile -> [128,102] stats.
    Host reduces 8 stats arrays -> 4 global maxes -> scales.
  - Launch 2: same matmuls; ACT tanh(scale*x) PSUM->SBUF, DVE relu,
    triu mask for tt, quantize to uint8, contiguous DMAs to the output.

The dominant cost end-to-end is the axon PJRT tunnel (~55-60 MB/s), so
the runner below (instead of run_bass_kernel_spmd) is built to minimize
host<->device traffic:
  - donated output buffers are created ON DEVICE (jnp.zeros under jit
    with out_shardings) instead of shipping host zeros up the tunnel;
  - the uv stack stash produced by launch 1 stays resident on device and
    is fed to launch 2 as a jax Array (no D2H+H2D roundtrip);
  - the output crosses the tunnel as uint8 (y in [0, tanh(1)]; quantized
    with step tanh(1)/255 -> l2 rel err ~5e-3) and is dequantized on host.
"""

import time

import numpy as np

import sys

sys.path.insert(0, "/opt/trn_rl_repo")

import jax
import jax.numpy as jnp
from jax.experimental.shard_map import shard_map
from jax.sharding import Mesh, NamedSharding, PartitionSpec

import concourse.bacc as bacc
import concourse.mybir as mybir
import concourse.tile as tile

F32 = mybir.dt.float32
F32R = mybir.dt.float32r
U8 = mybir.dt.uint8
Act = mybir.ActivationFunctionType
Alu = mybir.AluOpType
AxX = mybir.AxisListType.X

B, N, T, D = 16, 2048, 96, 32
S = N + T          # 2144
NC = 8             # cores
BPC = B // NC      # batches per core
P = 128
NBAND = N // P     # 16 spatial row-bands
EPS = 1e-30

TANH1 = float(np.tanh(1.0))
QK = 255.0 / TANH1          # quantization scale (device)
DEQ = TANH1 / 255.0         # dequantization scale (host)

# stats column layout, per batch (51 columns per batch)
_SS_COLS = list(range(0, 32))      # 16 bands x 2 half-tiles
_ST_COLS = list(range(32, 48))     # 16 bands
_TS_COLS = [48, 49]                # 2 half-tiles
_TT_COLS = [50]
NSTAT = 51 * BPC

EXEC_NS = {}


def _build(mode):
    """mode in ('max', 'out')."""
    assert mode in ("max", "out")
    nc = bacc.Bacc("TRN2", target_bir_lowering=False, debug=False, num_devices=NC)

    if mode == "out":
        uv_h = nc.dram_tensor("uv", [BPC, 2, P, S], F32R, kind="ExternalInput")
        scl_h = nc.dram_tensor("scl", [P, 4], F32, kind="ExternalInput")
        mask_h = nc.dram_tensor("mask", [T, T], F32, kind="ExternalInput")
        out_h = nc.dram_tensor("out", [BPC, S, S], U8, kind="ExternalOutput")
    else:
        embT_h = nc.dram_tensor("embT", [BPC, D, S], F32R, kind="ExternalInput")
        wp_h = nc.dram_tensor("Wpack", [D, 512], F32R, kind="ExternalInput")
        bias_h = nc.dram_tensor("biasp", [P, 4], F32, kind="ExternalInput")
        stats_h = nc.dram_tensor("stats", [P, NSTAT], F32, kind="ExternalOutput")
        uv_h = nc.dram_tensor("uv", [BPC, 2, P, S], F32R, kind="ExternalOutput")

    with tile.TileContext(nc) as tc:
        with (
            tc.tile_pool(name="const", bufs=1) as constp,
            tc.tile_pool(name="uv", bufs=2) as uvp,
            tc.tile_pool(name="stage", bufs=3) as stagep,
            tc.tile_pool(name="psb", bufs=3, space="PSUM") as psb,
            tc.tile_pool(name="pss", bufs=2, space="PSUM") as pss,
        ):
            dma = nc.sync.dma_start

            if mode == "out":
                scl = constp.tile([P, 4], F32, tag="scl")
                dma(scl[:, :], scl_h.ap()[:, :])
                mask = constp.tile([T, T], F32, tag="mask")
                dma(mask[:, :], mask_h.ap()[:, :])
                out_ap = out_h.ap()
            else:
                wp = constp.tile([D, 512], F32R, tag="wp")
                wpr = wp
                dma(wp[:, :], wp_h.ap()[:, :])
                biasp = constp.tile([P, 4], F32, tag="biasp")
                dma(biasp[:, :], bias_h.ap()[:, :])
                stats = constp.tile([P, NSTAT], F32, tag="stats")
                nc.vector.memset(stats[:, :], 0.0)

            for b in range(BPC):
                sbase = 51 * b
                U = uvp.tile([P, S], F32R, tag="U")
                V = uvp.tile([P, S], F32R, tag="V")
                if mode == "out":
                    # reuse the stacks stashed by the max launch
                    dma(U[:, :], uv_h.ap()[b, 0])
                    dma(V[:, :], uv_h.ap()[b, 1])
                else:
                    dma(U[0:D, :], embT_h.ap()[b])
                    dma(V[0:D, :], embT_h.ap()[b])

                    # ---- spatial linears: fill bands 1..3 of U and V ----
                    for h in range(2):
                        hh = 1024 * h
                        for wofs, dst, bcol in ((0, U, 0), (128, V, 1)):
                            ps = psb.tile([P, 1024], F32, tag="ps")
                            for q in range(2):
                                c0 = hh + 512 * q
                                nc.tensor.matmul(
                                    ps[:, 512 * q : 512 * q + 512],
                                    wpr[0:D, wofs : wofs + 128],
                                    U[0:D, c0 : c0 + 512],
                                    start=True,
                                    stop=True,
                                )
                            nc.scalar.activation(
                                dst[32:64, hh : hh + 1024], ps[32:64, :], Act.Tanh
                            )
                            nc.scalar.activation(
                                dst[64:96, hh : hh + 1024], ps[64:96, :], Act.Tanh
                            )
                            nc.scalar.activation(
                                dst[96:128, hh : hh + 1024],
                                ps[96:128, :],
                                Act.Identity,
                                bias=biasp[96:128, bcol : bcol + 1],
                            )
                            if dst is U:
                                nc.vector.tensor_scalar_mul(
                                    U[32:64, hh : hh + 1024],
                                    U[32:64, hh : hh + 1024], 3.0,
                                )
                                nc.vector.tensor_scalar_mul(
                                    U[64:96, hh : hh + 1024],
                                    U[64:96, hh : hh + 1024], -3.0,
                                )

                    # ---- temporal linears: band 3 cols 2048:2144 --------
                    for wofs, dst, bcol in ((256, U, 2), (384, V, 3)):
                        psq = pss.tile([P, T], F32, tag="pst")
                        nc.tensor.matmul(
                            psq[:, :],
                            wp[0:D, wofs : wofs + 128],
                            U[0:D, N:S],
                            start=True,
                            stop=True,
                        )
                        nc.scalar.activation(
                            dst[96:128, N:S],
                            psq[96:128, :],
                            Act.Identity,
                            bias=biasp[96:128, bcol : bcol + 1],
                        )
                        # psq rows 32:96 are exactly 0 (zero weight cols):
                        # writes f32r zeros so K=128 st/ts skip bands 1-2
                        nc.scalar.activation(dst[32:64, N:S], psq[32:64, :], Act.Tanh)
                        nc.scalar.activation(dst[64:96, N:S], psq[64:96, :], Act.Tanh)

                    # stash the finished stacks for the out launch
                    dma(uv_h.ap()[b, 0], U[:, :])
                    dma(uv_h.ap()[b, 1], V[:, :])

                # ---- spatial row-bands ----------------------------------
                for r in range(NBAND):
                    r0 = r * P
                    if mode == "out":
                        stage = stagep.tile([P, S], F32, tag="stage")
                        stq = stagep.tile([P, S], U8, tag="stq")
                    for h in range(2):
                        hh = 1024 * h
                        ps = psb.tile([P, 1024], F32, tag="ps")
                        for q in range(2):
                            c0 = hh + 512 * q
                            nc.tensor.matmul(
                                ps[:, 512 * q : 512 * q + 512],
                                U[0:96, r0 : r0 + P],
                                V[0:96, c0 : c0 + 512],
                                start=True,
                                stop=True,
                            )
                        if mode == "max":
                            c = sbase + 2 * r + h
                            nc.vector.tensor_reduce(
                                stats[:, c : c + 1], ps[:, :], AxX, Alu.max
                            )
                        else:
                            nc.scalar.activation(
                                stage[:, hh : hh + 1024],
                                ps[:, :],
                                Act.Tanh,
                                scale=scl[:, 0:1],
                            )
                    # st columns
                    pstt = pss.tile([P, T], F32, tag="pst")
                    nc.tensor.matmul(
                        pstt[:, :], U[:, r0 : r0 + P], V[:, N:S],
                        start=True, stop=True,
                    )
                    if mode == "max":
                        c = sbase + 32 + r
                        nc.vector.tensor_reduce(
                            stats[:, c : c + 1], pstt[:, :], AxX, Alu.max
                        )
                    else:
                        nc.scalar.activation(
                            stage[:, N:S], pstt[:, :], Act.Tanh, scale=scl[:, 1:2]
                        )
                        nc.vector.tensor_scalar_max(stage[:, :], stage[:, :], 0.0)
                        # f32->u8 cast is RNE: err in [-0.5, 0.5] quant steps
                        nc.vector.tensor_scalar_mul(stq[:, :], stage[:, :], QK)
                        dma(out_ap[b, r0 : r0 + P, :], stq[:, :])

                # ---- temporal row-band (ts | tt) ------------------------
                if mode == "out":
                    stage = stagep.tile([P, S], F32, tag="stage")
                    stq = stagep.tile([P, S], U8, tag="stq")
                for h in range(2):
                    hh = 1024 * h
                    ps = psb.tile([P, 1024], F32, tag="ps")
                    for q in range(2):
                        c0 = hh + 512 * q
                        nc.tensor.matmul(
                            ps[0:T, 512 * q : 512 * q + 512],
                            U[:, N:S],
                            V[:, c0 : c0 + 512],
                            start=True, stop=True,
                        )
                    if mode == "max":
                        c = sbase + 48 + h
                        nc.vector.tensor_reduce(
                            stats[0:T, c : c + 1], ps[0:T, :], AxX, Alu.max
                        )
                    else:
                        nc.scalar.activation(
                            stage[0:T, hh : hh + 1024],
                            ps[0:T, :],
                            Act.Tanh,
                            scale=scl[0:T, 2:3],
                        )
                pstt = pss.tile([P, T], F32, tag="pst")
                nc.tensor.matmul(
                    pstt[0:T, :], U[0:D, N:S], V[0:D, N:S], start=True, stop=True
                )
                if mode == "max":
                    c = sbase + 50
                    nc.vector.tensor_reduce(
                        stats[0:T, c : c + 1], pstt[0:T, :], AxX, Alu.max
                    )
                else:
                    nc.scalar.activation(
                        stage[0:T, N:S], pstt[0:T, :], Act.Tanh, scale=scl[0:T, 3:4]
                    )
                    nc.vector.tensor_scalar_max(
                        stage[0:T, :], stage[0:T, :], 0.0
                    )
                    nc.vector.tensor_tensor(
                        stage[0:T, N:S], stage[0:T, N:S], mask[:, :], Alu.mult
                    )
                    nc.vector.tensor_scalar_mul(stq[0:T, :], stage[0:T, :], QK)
                    dma(out_ap[b, N:S, :], stq[0:T, :])

            if mode == "max":
                dma(stats_h.ap()[:, :], stats[:, :])

    nc.compile()
    return nc


def _build_fused():
    """Single-launch variant: local maxes -> cross-core AllReduce(max)
    collective -> reciprocal -> second matmul pass -> uint8 output.
    Avoids the stats/scales host roundtrip and the second dispatch.

    stats column layout (grouped by block for contiguous reduction):
      ss: [0,64)   col = 32*b + 2*r + h
      st: [64,96)  col = 64 + 16*b + r
      ts: [96,100) col = 96 + 2*b + h
      tt: [100,102) col = 100 + b
    """
    import concourse.bass_isa as bass_isa

    nc = bacc.Bacc("TRN2", target_bir_lowering=False, debug=False, num_devices=NC)

    embT_h = nc.dram_tensor("embT", [BPC, D, S], F32R, kind="ExternalInput")
    wp_h = nc.dram_tensor("Wpack", [D, 512], F32R, kind="ExternalInput")
    bias_h = nc.dram_tensor("biasp", [P, 4], F32, kind="ExternalInput")
    mask_h = nc.dram_tensor("mask", [T, T], F32, kind="ExternalInput")
    out_h = nc.dram_tensor("out", [BPC, S, S], U8, kind="ExternalOutput")

    with tile.TileContext(nc) as tc:
        with (
            tc.tile_pool(name="const", bufs=1) as constp,
            tc.tile_pool(name="uv", bufs=2) as uvp,
            tc.tile_pool(name="stage", bufs=3) as stagep,
            tc.tile_pool(name="psb", bufs=3, space="PSUM") as psb,
            tc.tile_pool(name="pss", bufs=2, space="PSUM") as pss,
            tc.tile_pool(name="dram", bufs=1, space="DRAM") as dramp,
        ):
            dma = nc.sync.dma_start

            wp = constp.tile([D, 512], F32R, tag="wp")
            dma(wp[:, :], wp_h.ap()[:, :])
            biasp = constp.tile([P, 4], F32, tag="biasp")
            dma(biasp[:, :], bias_h.ap()[:, :])
            mask = constp.tile([T, T], F32, tag="mask")
            dma(mask[:, :], mask_h.ap()[:, :])
            stats = constp.tile([P, 102], F32, tag="stats")
            nc.vector.memset(stats[:, :], 0.0)
            out_ap = out_h.ap()

            # ---- build U/V stacks for both batches (stay in SBUF) ----
            Us, Vs = [], []
            for b in range(BPC):
                U = uvp.tile([P, S], F32R, tag="U")
                V = uvp.tile([P, S], F32R, tag="V")
                dma(U[0:D, :], embT_h.ap()[b])
                dma(V[0:D, :], embT_h.ap()[b])
                for h in range(2):
                    hh = 1024 * h
                    for wofs, dst, bcol in ((0, U, 0), (128, V, 1)):
                        ps = psb.tile([P, 1024], F32, tag="ps")
                        for q in range(2):
                            c0 = hh + 512 * q
                            nc.tensor.matmul(
                                ps[:, 512 * q : 512 * q + 512],
                                wp[0:D, wofs : wofs + 128],
                                U[0:D, c0 : c0 + 512],
                                start=True,
                                stop=True,
                            )
                        nc.scalar.activation(
                            dst[32:64, hh : hh + 1024], ps[32:64, :], Act.Tanh
                        )
                        nc.scalar.activation(
                            dst[64:96, hh : hh + 1024], ps[64:96, :], Act.Tanh
                        )
                        nc.scalar.activation(
                            dst[96:128, hh : hh + 1024],
                            ps[96:128, :],
                            Act.Identity,
                            bias=biasp[96:128, bcol : bcol + 1],
                        )
                        if dst is U:
                            nc.vector.tensor_scalar_mul(
                                U[32:64, hh : hh + 1024],
                                U[32:64, hh : hh + 1024], 3.0,
                            )
                            nc.vector.tensor_scalar_mul(
                                U[64:96, hh : hh + 1024],
                                U[64:96, hh : hh + 1024], -3.0,
                            )
                for wofs, dst, bcol in ((256, U, 2), (384, V, 3)):
                    psq = pss.tile([P, T], F32, tag="pst")
                    nc.tensor.matmul(
                        psq[:, :],
                        wp[0:D, wofs : wofs + 128],
                        U[0:D, N:S],
                        start=True,
                        stop=True,
                    )
                    nc.scalar.activation(
                        dst[96:128, N:S],
                        psq[96:128, :],
                        Act.Identity,
                        bias=biasp[96:128, bcol : bcol + 1],
                    )
                    nc.scalar.activation(dst[32:64, N:S], psq[32:64, :], Act.Tanh)
                    nc.scalar.activation(dst[64:96, N:S], psq[64:96, :], Act.Tanh)
                Us.append(U)
                Vs.append(V)

            def emit_blocks(b, phase, scl=None):
                U, V = Us[b], Vs[b]
                for r in range(NBAND):
                    r0 = r * P
                    if phase == "out":
                        stage = stagep.tile([P, S], F32, tag="stage")
                        stq = stagep.tile([P, S], U8, tag="stq")
                    for h in range(2):
                        hh = 1024 * h
                        ps = psb.tile([P, 1024], F32, tag="ps")
                        for q in range(2):
                            c0 = hh + 512 * q
                            nc.tensor.matmul(
                                ps[:, 512 * q : 512 * q + 512],
                                U[0:96, r0 : r0 + P],
                                V[0:96, c0 : c0 + 512],
                                start=True,
                                stop=True,
                            )
                        if phase == "max":
                            c = 32 * b + 2 * r + h
                            nc.vector.tensor_reduce(
                                stats[:, c : c + 1], ps[:, :], AxX, Alu.max
                            )
                        else:
                            nc.scalar.activation(
                                stage[:, hh : hh + 1024],
                                ps[:, :],
                                Act.Tanh,
                                scale=scl[:, 0:1],
                            )
                    pstt = pss.tile([P, T], F32, tag="pst")
                    nc.tensor.matmul(
                        pstt[:, :], U[:, r0 : r0 + P], V[:, N:S],
                        start=True, stop=True,
                    )
                    if phase == "max":
                        c = 64 + 16 * b + r
                        nc.vector.tensor_reduce(
                            stats[:, c : c + 1], pstt[:, :], AxX, Alu.max
                        )
                    else:
                        nc.scalar.activation(
                            stage[:, N:S], pstt[:, :], Act.Tanh, scale=scl[:, 1:2]
                        )
                        nc.vector.tensor_scalar_max(stage[:, :], stage[:, :], 0.0)
                        nc.vector.tensor_scalar_mul(stq[:, :], stage[:, :], QK)
                        dma(out_ap[b, r0 : r0 + P, :], stq[:, :])

                if phase == "out":
                    stage = stagep.tile([P, S], F32, tag="stage")
                    stq = stagep.tile([P, S], U8, tag="stq")
                for h in range(2):
                    hh = 1024 * h
                    ps = psb.tile([P, 1024], F32, tag="ps")
                    for q in range(2):
                        c0 = hh + 512 * q
                        nc.tensor.matmul(
                            ps[0:T, 512 * q : 512 * q + 512],
                            U[:, N:S],
                            V[:, c0 : c0 + 512],
                            start=True, stop=True,
                        )
                    if phase == "max":
                        c = 96 + 2 * b + h
                        nc.vector.tensor_reduce(
                            stats[0:T, c : c + 1], ps[0:T, :], AxX, Alu.max
                        )
                    else:
                        nc.scalar.activation(
                            stage[0:T, hh : hh + 1024],
                            ps[0:T, :],
                            Act.Tanh,
                            scale=scl[0:T, 2:3],
                        )
                pstt = pss.tile([P, T], F32, tag="pst")
                nc.tensor.matmul(
                    pstt[0:T, :], U[0:D, N:S], V[0:D, N:S], start=True, stop=True
                )
                if phase == "max":
                    c = 100 + b
                    nc.vector.tensor_reduce(
                        stats[0:T, c : c + 1], pstt[0:T, :], AxX, Alu.max
                    )
                else:
                    nc.scalar.activation(
                        stage[0:T, N:S], pstt[0:T, :], Act.Tanh, scale=scl[0:T, 3:4]
                    )
                    nc.vector.tensor_scalar_max(stage[0:T, :], stage[0:T, :], 0.0)
                    nc.vector.tensor_tensor(
                        stage[0:T, N:S], stage[0:T, N:S], mask[:, :], Alu.mult
                    )
                    nc.vector.tensor_scalar_mul(stq[0:T, :], stage[0:T, :], QK)
                    dma(out_ap[b, N:S, :], stq[0:T, :])

            # ---- pass 1: local maxes ------------------------------------
            for b in range(BPC):
                emit_blocks(b, "max")

            # ---- global max: partition reduce + cross-core AllReduce ----
            red = constp.tile([P, 4], F32, tag="red")
            nc.vector.tensor_reduce(red[:, 0:1], stats[:, 0:64], AxX, Alu.max)
            nc.vector.tensor_reduce(red[:, 1:2], stats[:, 64:96], AxX, Alu.max)
            nc.vector.tensor_reduce(red[:, 2:3], stats[:, 96:100], AxX, Alu.max)
            nc.vector.tensor_reduce(red[:, 3:4], stats[:, 100:102], AxX, Alu.max)
            redb = constp.tile([P, 4], F32, tag="redb")
            nc.gpsimd.partition_all_reduce(
                redb[:, :], red[:, :], 128, bass_isa.ReduceOp.max
            )
            ccin = dramp.tile([P, 4], F32, tag="ccin")
            ccout = dramp.tile([P, 4], F32, tag="ccout")
            nc.gpsimd.dma_start(ccin[:, :], redb[:, :])
            nc.gpsimd.collective_compute(
                "AllReduce",
                Alu.max,
                replica_groups=[list(range(NC))],
                ins=[ccin.opt()],
                outs=[ccout.opt()],
            )
            gmax = constp.tile([P, 4], F32, tag="gmax")
            nc.gpsimd.dma_start(gmax[:, :], ccout[:, :])
            nc.vector.tensor_scalar_add(gmax[:, :], gmax[:, :], EPS)
            scl = constp.tile([P, 4], F32, tag="scl")
            nc.vector.reciprocal(scl[:, :], gmax[:, :])

            # ---- pass 2: normalized output ------------------------------
            for b in range(BPC):
                emit_blocks(b, "out", scl=scl)

    nc.compile()
    return nc


class _Exec:
    """SPMD executor for a compiled Bass program over the 8 axon cores.

    Differences vs run_bass_kernel_spmd (both matter a lot on the slow
    axon tunnel): donated output zero-buffers are created on device, and
    inputs/outputs are jax Arrays so intermediates can stay on device
    between launches.
    """

    def __init__(self, nc):
        from concourse.bass2jax import (
            _bass_exec_p,
            install_neuronx_cc_hook,
            partition_id_tensor,
        )

        install_neuronx_cc_hook()
        assert nc.dbg_addr is None or not nc.dbg_callbacks

        partition_name = (
            nc.partition_id_tensor.name if nc.partition_id_tensor else None
        )
        in_names: list[str] = []
        out_names: list[str] = []
        out_avals: list[jax.core.ShapedArray] = []
        zinfo: list[tuple[tuple, np.dtype]] = []
        for alloc in nc.m.functions[0].allocations:
            if not isinstance(alloc, mybir.MemoryLocationSet):
                continue
            assert alloc.memorylocations
            name = alloc.memorylocations[0].name
            if alloc.kind == "ExternalInput":
                if name != partition_name:
                    in_names.append(name)
            elif alloc.kind == "ExternalOutput":
                assert alloc.tensor_shape is not None and alloc.dtype is not None
                shape = tuple(alloc.tensor_shape)
                dtype = mybir.dt.np(alloc.dtype)
                out_names.append(name)
                out_avals.append(jax.core.ShapedArray(shape, dtype))
                zinfo.append(((NC * shape[0], *shape[1:]), dtype))

        self.param_names = list(in_names)
        self.out_names = out_names
        n_params = len(in_names)
        n_outs = len(out_names)
        all_names = in_names + out_names
        if partition_name is not None:
            all_names = all_names + [partition_name]
        donate = tuple(range(n_params, n_params + n_outs))

        def _body(*args):
            operands = list(args)
            if partition_name is not None:
                operands.append(partition_id_tensor())
            outs = _bass_exec_p.bind(
                *operands,
                out_avals=tuple(out_avals),
                in_names=tuple(all_names),
                out_names=tuple(out_names),
                lowering_input_output_aliases=(),
                sim_require_finite=True,
                sim_require_nnan=True,
                nc=nc,
            )
            return tuple(outs)

        devices = jax.devices()[:NC]
        assert len(devices) == NC
        mesh = Mesh(np.asarray(devices), ("core",))
        in_specs = (PartitionSpec("core"),) * (n_params + n_outs)
        out_specs = (PartitionSpec("core"),) * n_outs
        self.sharded = jax.jit(
            shard_map(
                _body,
                mesh=mesh,
                in_specs=in_specs,
                out_specs=out_specs,
                check_rep=False,
            ),
            donate_argnums=donate,
            keep_unused=True,
        )
        sh = NamedSharding(mesh, PartitionSpec("core"))
        self.zeros_fn = jax.jit(
            lambda: tuple(jnp.zeros(s, d) for s, d in zinfo),
            out_shardings=(sh,) * n_outs if n_outs > 1 else sh,
        )

    def __call__(self, gin: dict) -> dict:
        """gin: name -> global array ([NC*dim0, ...]; np or jax Array)."""
        args = [gin[n] for n in self.param_names]
        zeros = self.zeros_fn()
        if not isinstance(zeros, tuple):
            zeros = (zeros,)
        outs = self.sharded(*args, *zeros)
        return dict(zip(self.out_names, outs))


_PROGS = {}


def _prog(mode):
    if mode not in _PROGS:
        nc = _build_fused() if mode == "fused" else _build(mode)
        _PROGS[mode] = _Exec(nc)
    return _PROGS[mode]


def _host_pack(inputs):
    s = np.asarray(inputs["spatial_nodes"], dtype=np.float32)
    t = np.asarray(inputs["temporal_nodes"], dtype=np.float32)
    emb = np.concatenate([s, t], axis=1)                    # [B, S, D]
    embT = np.ascontiguousarray(emb.transpose(0, 2, 1))     # [B, D, S]

    wp = np.zeros((D, 512), dtype=np.float32)
    # U bands: 1 -> n1=tanh(3 s W1^T) (x3 later), 2 -> n2 (x-3 later), 3 -> q_st
    wp[:, 32:64] = (3.0 * np.asarray(inputs["W_ss1"])).T
    wp[:, 64:96] = (3.0 * np.asarray(inputs["W_ss2"])).T
    wp[:, 96:128] = np.asarray(inputs["Wq_st"]).T
    # V bands: 1 -> n2, 2 -> n1, 3 -> k_ts
    wp[:, 160:192] = (3.0 * np.asarray(inputs["W_ss2"])).T
    wp[:, 192:224] = (3.0 * np.asarray(inputs["W_ss1"])).T
    wp[:, 224:256] = np.asarray(inputs["Wk_ts"]).T
    # temporal: U band3 -> q_ts ; V band3 -> k_st
    wp[:, 352:384] = np.asarray(inputs["Wq_ts"]).T
    wp[:, 480:512] = np.asarray(inputs["Wk_st"]).T

    biasp = np.zeros((P, 4), dtype=np.float32)
    biasp[96:128, 0] = np.asarray(inputs["bq_st"])
    biasp[96:128, 1] = np.asarray(inputs["bk_ts"])
    biasp[96:128, 2] = np.asarray(inputs["bq_ts"])
    biasp[96:128, 3] = np.asarray(inputs["bk_st"])

    mask = np.triu(np.ones((T, T), dtype=np.float32))
    return embT, wp, biasp, mask


def _dequant(q):
    out = np.empty(q.shape, dtype=np.float32)
    np.multiply(q, np.float32(DEQ), out=out)
    return out


FUSED = True


def kernel(profile=False, **inputs):
    if FUSED:
        try:
            return _kernel_fused(**inputs)
        except Exception as e:  # fall back to the 2-launch path
            print(f"fused kernel failed ({type(e).__name__}: {e}); split", flush=True)
    return _kernel_split(**inputs)


def _kernel_fused(**inputs):
    tt0 = time.monotonic()
    embT, wp, biasp, mask = _host_pack(inputs)
    tt1 = time.monotonic()
    EXEC_NS["pack_wall"] = (tt1 - tt0) * 1e9

    ex = _prog("fused")
    gin = {
        "embT": embT,                       # [16, D, S] == NC x [BPC, D, S]
        "Wpack": np.tile(wp, (NC, 1)),
        "biasp": np.tile(biasp, (NC, 1)),
        "mask": np.tile(mask, (NC, 1)),
    }
    EXEC_NS["max"] = None
    EXEC_NS["max_wall"] = 0.0
    t0 = time.monotonic()
    o = ex(gin)
    q = np.asarray(o["out"])                # [16, S, S] uint8 over the tunnel
    t1 = time.monotonic()
    EXEC_NS["out"] = None
    EXEC_NS["out_wall"] = (t1 - t0) * 1e9
    out = _dequant(q)
    EXEC_NS["deq_wall"] = (time.monotonic() - t1) * 1e9
    return out


def _kernel_split(**inputs):
    tt0 = time.monotonic()
    embT, wp, biasp, mask = _host_pack(inputs)
    tt1 = time.monotonic()
    EXEC_NS["pack_wall"] = (tt1 - tt0) * 1e9

    ex1 = _prog("max")
    gin1 = {
        "embT": embT,                       # [16, D, S] == NC x [BPC, D, S]
        "Wpack": np.tile(wp, (NC, 1)),
        "biasp": np.tile(biasp, (NC, 1)),
    }
    t0 = time.monotonic()
    o1 = ex1(gin1)
    stats = np.asarray(o1["stats"]).reshape(NC, P, NSTAT)
    t1 = time.monotonic()
    EXEC_NS["max"] = None
    EXEC_NS["max_wall"] = (t1 - t0) * 1e9

    cols = {
        "ss": [51 * b + c for b in range(BPC) for c in _SS_COLS],
        "st": [51 * b + c for b in range(BPC) for c in _ST_COLS],
        "ts": [51 * b + c for b in range(BPC) for c in _TS_COLS],
        "tt": [51 * b + c for b in range(BPC) for c in _TT_COLS],
    }
    scales = np.zeros((P, 4), dtype=np.float32)
    for j, blk in enumerate(("ss", "st", "ts", "tt")):
        m = float(stats[:, :, cols[blk]].max())  # stats memset to 0 -> m >= 0
        scales[:, j] = np.float32(1.0 / (m + EPS))

    ex2 = _prog("out")
    gin2 = {
        "uv": o1["uv"],                     # stays on device
        "scl": np.tile(scales, (NC, 1)),
        "mask": np.tile(mask, (NC, 1)),
    }
    t0 = time.monotonic()
    o2 = ex2(gin2)
    q = np.asarray(o2["out"])               # [16, S, S] uint8 over the tunnel
    t1 = time.monotonic()
    EXEC_NS["out"] = None
    EXEC_NS["out_wall"] = (t1 - t0) * 1e9
    out = _dequant(q)
    EXEC_NS["deq_wall"] = (time.monotonic() - t1) * 1e9
    return out
